# revision 5
# baseline (speedup 1.0000x reference)
"""Trainium2 Bass kernel for the histogram-binning CTC-style loss.

reference:
    x: [T=512, B=32, V=10000] f32, label: [B=32, L=64] int
    counts[b,v] = histogram of non-blank labels; counts[b,0] = T - len_b
    loss = -sum_{b,v} (counts[b,v]/T) * log(mean_t x[t,b,v] + 1e-10) / B

Strategy (8 NeuronCores, data-parallel over batch, sparse gather):
    Only columns v with counts[b,v] != 0 contribute: <=64 unique labels +
    the blank per batch -> <=65 columns x 4 local batches = 260 columns
    per core out of 40000.  Each core receives a per-core COLUMN TABLE
    (int32 data) and WEIGHTS; the device reads each table entry into a
    sequencer register and issues a dynamic-offset DMA for that column
    ([128 t-partitions x 4 t-chunks x 1col]), round-robined over the three
    DMA-generation rings (sync/scalar HWDGE + gpsimd SWDGE).  The T-sum is
    a ones-vector matmul accumulated over the 4 t-chunks in PSUM, then
    ScalarE Ln (scale=1/T, bias=1e-10), VectorE dot with the weights, and
    a single partial scalar out per core; the host sums the 8 partials.

    One program serves any input: column tables are data, not code.
"""

import numpy as np

import concourse.bass as bass
import concourse.bacc as bacc
import concourse.mybir as mybir
import concourse.tile as tile
from concourse.bass_utils import run_bass_kernel_spmd

T = 512
B = 32
V = 10000
L = 64
NCORES = 8
BL = B // NCORES          # local batches per core
F = BL * V                # flattened (b, v) columns per core
TCH = T // 128            # T chunks of 128 partitions
PER = 65                  # column slots per batch (<=64 labels + blank)
NCOLS = BL * PER          # 260 gathered columns per core

_NC_CACHE = {}


def _build_nc(rings=("sync", "scalar", "gpsimd"), interleave=2, loop=1):
    n = NCOLS
    nc = bacc.Bacc()
    x_d = nc.declare_dram_parameter("x", [T, F], mybir.dt.float32, isOutput=False)
    c_d = nc.declare_dram_parameter("cols", [1, n], mybir.dt.int32, isOutput=False)
    w_d = nc.declare_dram_parameter("w", [1, n], mybir.dt.float32, isOutput=False)
    out_d = nc.declare_dram_parameter("out", [1, 1], mybir.dt.float32, isOutput=True)
    xr = x_d[:, :].rearrange("(c p) f -> p c f", p=128)  # [128, TCH, F]

    import contextlib
    with tile.TileContext(nc) as tc:
        loop_cm = tc.For_i(0, loop, 1) if loop > 1 else contextlib.nullcontext()
        with (
            loop_cm,
            tc.tile_pool(name="gp", bufs=1) as gp,
            tc.tile_pool(name="sp", bufs=1) as sp,
            tc.tile_pool(name="cp", bufs=1) as cp,
            tc.tile_pool(name="psum", bufs=1, space="PSUM") as psum,
        ):
            ones = cp.tile([128, 1], mybir.dt.bfloat16)
            nc.gpsimd.memset(ones[:], 1.0)
            biasv = cp.tile([1, 1], mybir.dt.float32)
            nc.gpsimd.memset(biasv[:], 1e-10)

            ct = cp.tile([1, n], mybir.dt.int32)
            nc.sync.dma_start(out=ct[:], in_=c_d[:, :])
            wt = sp.tile([1, n], mybir.dt.float32)
            nc.sync.dma_start(out=wt[:], in_=w_d[:, :])

            engs = [{"sync": nc.sync, "scalar": nc.scalar, "gpsimd": nc.gpsimd}[r]
                    for r in rings]
            ne = len(engs)
            xg = gp.tile([128, TCH, n], mybir.dt.float32)
            slots = [[] for _ in range(ne)]
            for i in range(n):
                slots[i % ne].append(i)
            for e, eng in enumerate(engs):
                my = slots[e]
                regs = [contextlib.ExitStack() for _ in range(interleave)]
                rhandles = []
                for k in range(interleave):
                    r = regs[k].enter_context(eng.register(f"col_e{e}_{k}"))
                    rhandles.append(r)
                for base in range(0, len(my), interleave):
                    grp = my[base:base + interleave]
                    offs = []
                    for k, i in enumerate(grp):
                        eng.reg_load(rhandles[k], ct[0:1, i:i + 1])
                        offs.append(eng.snap(rhandles[k]))
                    for k, i in enumerate(grp):
                        eng.dma_start(out=xg[:, :, i:i + 1],
                                      in_=xr[:, :, bass.ds(offs[k], 1)])
                for k in range(interleave):
                    regs[k].close()

            xgb = sp.tile([128, TCH, n], mybir.dt.bfloat16)
            nc.vector.tensor_copy(xgb[:], xg[:])
            ps = psum.tile([1, n], mybir.dt.float32)
            for c in range(TCH):
                nc.tensor.matmul(
                    ps[:], ones[:], xgb[:, c, :],
                    start=(c == 0), stop=(c == TCH - 1),
                )
            logv = sp.tile([1, n], mybir.dt.float32)
            nc.scalar.activation(
                logv[:], ps[:], mybir.ActivationFunctionType.Ln,
                bias=biasv[:], scale=1.0 / T,
            )
            prod = sp.tile([1, n], mybir.dt.float32)
            nc.vector.tensor_tensor(
                out=prod[:], in0=logv[:], in1=wt[:], op=mybir.AluOpType.mult,
            )
            total = cp.tile([1, 1], mybir.dt.float32)
            nc.vector.tensor_reduce(
                out=total[:], in_=prod[:], axis=mybir.AxisListType.X,
                op=mybir.AluOpType.add,
            )
            nc.sync.dma_start(out=out_d[:, :], in_=total[:])

    nc.finalize()
    return nc


def get_nc():
    if "nc" not in _NC_CACHE:
        _NC_CACHE["nc"] = _build_nc()
    return _NC_CACHE["nc"]


def plan_core(label_rows):
    """label_rows: [BL, L] labels for one core's batches.
    Returns cols [NCOLS] int32 flattened (b*V + v) indices and w [1, NCOLS]
    f32 count weights (blank slot weight = T - len_b; padding weight 0)."""
    cols = np.zeros(NCOLS, dtype=np.int32)
    w = np.zeros((1, NCOLS), dtype=np.float32)
    for b in range(BL):
        lab = np.asarray(label_rows[b])
        m = lab != 0
        vals, cnts = np.unique(lab[m], return_counts=True)
        assert len(vals) <= PER - 1
        base = b * PER
        cols[base] = b * V + 0
        w[0, base] = T - m.sum()
        cols[base + 1:base + 1 + len(vals)] = b * V + vals
        w[0, base + 1:base + 1 + len(vals)] = cnts
        cols[base + 1 + len(vals):base + PER] = b * V  # pad: col 0, weight 0
    return cols, w


def make_in_maps(x, label):
    x = np.ascontiguousarray(np.asarray(x, dtype=np.float32))
    label = np.asarray(label)
    in_maps = []
    for c in range(NCORES):
        xs = np.ascontiguousarray(x[:, c * BL:(c + 1) * BL, :]).reshape(T, F)
        cols, w = plan_core(label[c * BL:(c + 1) * BL])
        in_maps.append({"x": xs, "cols": cols.reshape(1, -1), "w": w})
    return in_maps


def kernel(x, label):
    nc = get_nc()
    in_maps = make_in_maps(x, label)
    res = run_bass_kernel_spmd(nc, in_maps, core_ids=list(range(NCORES)))
    part = sum(float(res.results[c]["out"][0, 0]) for c in range(NCORES))
    loss = -part / (T * B)
    return np.float32(loss)
